# revision 14
# baseline (speedup 1.0000x reference)
"""Trainium2 Bass kernel for nn_CrossAttention (b=2, n=1024, L=4*1024, d=1024, h=8).

Sharding: 8 cores = 2 batches (data parallel) x 4 head-groups (tensor parallel,
2 heads each).  Each core computes Q/K/V projections for its head slice, the
attention, and a row-parallel partial of the output projection.  The host sums
the 4 partials per batch and adds the bias (the "all-reduce" happens at gather
time on the host).

Device-side compute (per core, all matmuls bf16 with fp32 PSUM accumulation):
  QT[dq,n]   = (Wq_g^T x_b^T)        via lhsT=Wq chunks,  rhs=xT chunks
  KT[dk,L]   = (Wk_g^T ctx_b^T)      via lhsT=Wk chunks,  rhs=ctxT chunks
  V[L,dv]    = (ctx_b Wv_g)          via lhsT=ctxT chunks, rhs=Wv
  ST[L,q]    = KT_h^T QT_h           (attention scores, transposed layout)
  E = exp(scale*ST + sim)            fused on ScalarE (sim const per 128-chunk)
  den[q]     = sum_L E               DVE halving tree + ones-matmul partition sum
  OT[hd,q]   = V_h^T E               accumulated over L chunks
  O          = OT * (1/den)          partition-broadcast reciprocal
  out[q,d]  += O_h^T Wout_g          row-parallel partial, fp32 out

Softmax max-subtraction is skipped: logits are bounded (|scale*qk| < ~1,
|sim| < ~4), so exp never overflows and the result is mathematically identical.
"""

import numpy as np
import ml_dtypes

P = 128
D = 1024
N = 1024          # queries per batch
L = 4096          # keys per batch (m * nc)
DC = D // P       # 8 contraction chunks
LT = L // P       # 32 L tiles
NH = 2            # heads per core
SCALE = float(D) ** -0.5

_CACHE = {}


def _build_program():
    import concourse.tile as tile
    from concourse import bacc, mybir

    nc = bacc.Bacc(
        "TRN2",
        target_bir_lowering=False,
        debug=False,
        enable_asserts=True,
        num_devices=8,
    )
    bf16 = mybir.dt.bfloat16
    f32 = mybir.dt.float32
    EXP = mybir.ActivationFunctionType.Exp

    # Host-prepared layouts: partition dim first, contraction chunk second.
    xT = nc.dram_tensor("xT", [P, DC, N], bf16, kind="ExternalInput")
    # ctxT arrives as 8 L-slices of 512 keys so K/V compute can start after
    # the first 1MB lands instead of after the full 8MB.
    ctxT = nc.dram_tensor("ctxT", [L // 512, P, DC, 512], bf16, kind="ExternalInput")
    wq = nc.dram_tensor("wq", [P, DC, NH * P], bf16, kind="ExternalInput")
    wk = nc.dram_tensor("wk", [P, DC, NH * P], bf16, kind="ExternalInput")
    wv = nc.dram_tensor("wv", [P, DC, NH * P], bf16, kind="ExternalInput")
    wout = nc.dram_tensor("wout", [P, NH, D], bf16, kind="ExternalInput")
    simb = nc.dram_tensor("simb", [P, LT], f32, kind="ExternalInput")
    out = nc.dram_tensor("out", [N // P, P, D], f32, kind="ExternalOutput")
    ones_dram = nc.inline_tensor(np.ones((P, 1), np.float32), name="ones_c")

    with tile.TileContext(nc) as tc:
        with (
            tc.tile_pool(name="res", bufs=1) as res,
            tc.tile_pool(name="ps_mm", bufs=2, space="PSUM") as ps_mm,
            tc.tile_pool(name="ps_acc", bufs=2, space="PSUM") as ps_acc,
        ):
            # ---- resident tiles (DMAs ordered by first use on the PE) ----------
            ones_sb = res.tile([P, 1], f32, tag="ones")
            sim_sb = res.tile([P, LT], f32, tag="sim")
            wq_sb = res.tile([P, DC, NH * P], bf16, tag="wq")
            wk_sb = res.tile([P, DC, NH * P], bf16, tag="wk")
            wv_sb = res.tile([P, DC, NH * P], bf16, tag="wv")
            wout_sb = res.tile([P, NH, D], bf16, tag="wout")

            qT_sb = res.tile([P, NH, N], bf16, tag="qT")
            kT_sb = res.tile([P, NH, L], bf16, tag="kT")
            v_sb = res.tile([P, LT, NH * P], bf16, tag="v")
            o_sb = res.tile([P, NH, N], bf16, tag="o")

            with tc.tile_pool(name="p1", bufs=1) as p1:
                # slice-major ctxT so each DMA slice lands contiguously
                ctxT_sb = p1.tile([P, L // 512, DC, 512], bf16, tag="ctxT")
                xT_sb = p1.tile([P, DC, N], bf16, tag="xT")

                def dma_ctx_slice(j):
                    nc.sync.dma_start(ctxT_sb[:, j, :, :], ctxT[j])

                nc.gpsimd.dma_start(ctxT_sb[:, 0, :, :], ctxT[0])
                nc.sync.dma_start(wk_sb[:], wk[:])
                nc.gpsimd.dma_start(wv_sb[:], wv[:])
                for j in range(1, L // 512):
                    dma_ctx_slice(j)
                nc.sync.dma_start(wq_sb[:], wq[:])
                nc.sync.dma_start(sim_sb[:], simb[:])
                nc.sync.dma_start(ones_sb[:], ones_dram[:])
                nc.sync.dma_start(xT_sb[:], xT[:])
                nc.sync.dma_start(wout_sb[:], wout[:])

                # ---- phase 1: KT[dk, L] slice by slice, then QT ---------------
                for j in range(L // 512):
                    lo = j * 512
                    for h in range(NH):
                        ps = ps_mm.tile([P, 2, 512], f32, tag="mm")
                        for c in range(DC):
                            nc.tensor.matmul(
                                ps[:, 0, :],
                                lhsT=wk_sb[:, c, h * P:(h + 1) * P],
                                rhs=ctxT_sb[:, j, c, :],
                                start=(c == 0),
                                stop=(c == DC - 1),
                            )
                        nc.vector.tensor_copy(
                            out=kT_sb[:, h, lo:lo + 512], in_=ps[:, 0, :]
                        )

                for h in range(NH):
                    ps = ps_mm.tile([P, 2, 512], f32, tag="mm")
                    for half in range(2):
                        for c in range(DC):
                            nc.tensor.matmul(
                                ps[:, half, :],
                                lhsT=wq_sb[:, c, h * P:(h + 1) * P],
                                rhs=xT_sb[:, c, half * 512:(half + 1) * 512],
                                start=(c == 0),
                                stop=(c == DC - 1),
                            )
                    nc.vector.tensor_copy(
                        out=qT_sb[:, h, :].rearrange("p (a b) -> p a b", a=2),
                        in_=ps[:],
                    )

                # V group t: V[L-tile t, dv] accumulated over 8 d-chunks.
                # Emitted inside the h==0 attention loop: phase 2's exp work
                # exceeds its S/PV matmul work, so V fills the PE slack there.
                def emit_v_group(t):
                    ps = ps_acc.tile([P, 2, 512], f32, tag="acc")
                    for c in range(DC):
                        nc.tensor.matmul(
                            ps[:, 0, :NH * P],
                            lhsT=ctxT_sb[:, t // 4, c, (t % 4) * P:(t % 4 + 1) * P],
                            rhs=wv_sb[:, c, :],
                            start=(c == 0),
                            stop=(c == DC - 1),
                        )
                    nc.vector.tensor_copy(out=v_sb[:, t, :], in_=ps[:, 0, :NH * P])

                # ---- phase 2: streaming attention -----------------------------
                with tc.tile_pool(name="p2", bufs=4) as p2:
                    with tc.tile_pool(name="p2acc", bufs=2) as p2acc:
                        recip_sb = p2acc.tile([P, 512], f32, tag="recip")
                        recipb_sb = p2acc.tile([P, 2, 512], f32, tag="recipb")
                        po = {}

                        def emit_norm(h, qt):
                            # partition-reduce the denominator, reciprocal,
                            # broadcast, scale P@V -> normalized O^T (bf16)
                            dps = ps_acc.tile([P, 2, 512], f32, tag="acc")
                            nc.tensor.matmul(
                                dps[:1, qt, :],
                                lhsT=ones_sb[:, :1],
                                rhs=den_e[:, qt, :],
                                start=True,
                                stop=True,
                            )
                            nc.vector.reciprocal(recip_sb[:1, :], dps[:1, qt, :])
                            nc.gpsimd.partition_broadcast(
                                recipb_sb[:, qt, :], recip_sb[:1, :]
                            )
                            nc.vector.tensor_mul(
                                out=o_sb[:, h, qt * 512:(qt + 1) * 512],
                                in0=po[h][:, qt, :],
                                in1=recipb_sb[:, qt, :],
                            )

                        def emit_out_tile(t8):
                            ps = ps_mm.tile([P, 2, 512], f32, tag="mm")
                            for nt in range(2):
                                for h in range(NH):
                                    nc.tensor.matmul(
                                        ps[:, nt, :],
                                        lhsT=o_sb[:, h, t8 * P:(t8 + 1) * P],
                                        rhs=wout_sb[:, h, nt * 512:(nt + 1) * 512],
                                        start=(h == 0),
                                        stop=(h == NH - 1),
                                    )
                            osb = p3.tile([P, 2, 512], f32, tag="osb")
                            # alternate engines so the PSUM->SBUF drain
                            # doesn't pace the kernel tail on DVE alone
                            if t8 % 2 == 0:
                                nc.vector.tensor_copy(out=osb[:], in_=ps[:])
                            else:
                                nc.scalar.copy(out=osb[:], in_=ps[:])
                            nc.sync.dma_start(
                                out[t8].rearrange("p (a b) -> p a b", a=2), osb[:]
                            )

                        for h in range(NH):
                            po[h] = ps_acc.tile([P, 2, 512], f32, tag="acc", name=f"po{h}")
                            # denominator accumulates on two engines in
                            # parallel chains: GpSimd ops are ~2x DVE cost,
                            # so DVE takes 2 of every 3 chunks.
                            den_e = p2acc.tile([P, 2, 512], f32, tag="den_e")
                            den_o = p2acc.tile([P, 2, 512], f32, tag="den_o")
                            exp_tiles = []
                            for t in range(LT):
                                ps = ps_mm.tile([P, 2, 512], f32, tag="mm")
                                for qt in range(2):
                                    nc.tensor.matmul(
                                        ps[:, qt, :],
                                        lhsT=kT_sb[:, h, t * P:(t + 1) * P],
                                        rhs=qT_sb[:, h, qt * 512:(qt + 1) * 512],
                                        start=True,
                                        stop=True,
                                    )
                                expS = p2.tile([P, 2, 512], bf16, tag="expS")
                                exp_tiles.append(expS)
                                nc.scalar.activation(
                                    out=expS[:],
                                    in_=ps[:],
                                    func=EXP,
                                    bias=sim_sb[:, t:t + 1],
                                    scale=SCALE,
                                )
                                # P@V lags one chunk so the PE never waits on
                                # the exp that was just issued.
                                if t > 0:
                                    eprev = exp_tiles[t - 1]
                                    for qt in range(2):
                                        nc.tensor.matmul(
                                            po[h][:, qt, :],
                                            lhsT=v_sb[:, t - 1, h * P:(h + 1) * P],
                                            rhs=eprev[:, qt, :],
                                            start=(t == 1),
                                            stop=False,
                                        )
                                if h == 0:
                                    emit_v_group(t)
                                on_dve = t % 3 != 2
                                eng = nc.vector if on_dve else nc.gpsimd
                                den = den_e if on_dve else den_o
                                first = t == 0 if on_dve else t == 2
                                if first:
                                    eng.tensor_copy(out=den[:], in_=expS[:])
                                else:
                                    eng.tensor_add(out=den[:], in0=den[:], in1=expS[:])
                            for qt in range(2):
                                nc.tensor.matmul(
                                    po[h][:, qt, :],
                                    lhsT=v_sb[:, LT - 1, h * P:(h + 1) * P],
                                    rhs=exp_tiles[LT - 1][:, qt, :],
                                    start=False,
                                    stop=True,
                                )
                            nc.vector.tensor_add(
                                out=den_e[:], in0=den_e[:], in1=den_o[:]
                            )

                            if h == 0:
                                for qt in range(2):
                                    emit_norm(0, qt)

                        # h==1: interleave the output projection with the two
                        # normalize chains so the PE starts phase 3 as soon as
                        # the first q half is normalized.
                        with tc.tile_pool(name="p3", bufs=4) as p3:
                            emit_norm(1, 0)
                            for t8 in range(4):
                                emit_out_tile(t8)
                            emit_norm(1, 1)
                            for t8 in range(4, N // P):
                                emit_out_tile(t8)

    nc.compile()
    return nc


def _get_program():
    if "nc" not in _CACHE:
        _CACHE["nc"] = _build_program()
    return _CACHE["nc"]


def _prep_core_inputs(x, context, doc_similarities, Wq, Wkv, beta, Wout, core):
    bf = ml_dtypes.bfloat16
    b, g = divmod(core, 4)
    gs = slice(g * NH * P, (g + 1) * NH * P)

    def chunked(a2d, free):  # [D, free] -> [P, DC, free], partition-major
        return np.ascontiguousarray(
            a2d.astype(bf).reshape(DC, P, free).transpose(1, 0, 2)
        )

    xT_h = chunked(x[b].T, N)
    # [j, p, c, l]: L-slice-major so each slice is one contiguous DMA
    ctxT_h = np.ascontiguousarray(
        context[b].reshape(L, D).T.astype(bf)
        .reshape(DC, P, L // 512, 512).transpose(2, 1, 0, 3)
    )
    wq_h = chunked(Wq[:, gs], NH * P)
    wk_h = chunked(Wkv[:, gs], NH * P)
    wv_h = chunked(Wkv[:, D:][:, gs], NH * P)
    wout_h = np.ascontiguousarray(
        Wout[gs, :].astype(bf).reshape(NH, P, D).transpose(1, 0, 2)
    )
    sim = np.repeat(
        np.asarray(doc_similarities[b], np.float32) * np.float32(beta), L // 4
    )
    sim_h = np.ascontiguousarray(sim.reshape(LT, P).T.astype(np.float32))
    return {
        "xT": xT_h,
        "ctxT": ctxT_h,
        "wq": wq_h,
        "wk": wk_h,
        "wv": wv_h,
        "wout": wout_h,
        "simb": sim_h,
    }


def kernel(x, context, doc_similarities, Wq, Wkv, beta, Wout, bout):
    from concourse.bass_utils import run_bass_kernel_spmd

    nc = _get_program()
    in_maps = [
        _prep_core_inputs(x, context, doc_similarities, Wq, Wkv, beta, Wout, core)
        for core in range(8)
    ]
    res = run_bass_kernel_spmd(nc, in_maps, list(range(8)))
    full = np.zeros((2, N, D), np.float32)
    for core in range(8):
        full[core // 4] += res.results[core]["out"].reshape(N, D)
    full += np.asarray(bout, np.float32)[None, None, :]
    return full
